# revision 15
# baseline (speedup 1.0000x reference)
"""CarFormer attention kernel for 8 TRN2 NeuronCores.

Sharding: data-parallel over batch (B=2), tensor-parallel over heads
(16 heads -> 4 per core). Core c handles batch b=c//4, heads
[4*(c%4), 4*(c%4)+4). Each core computes q/k/v projections for its head
slice, RoPE, causal flash-style attention (transposed-scores layout), and
a partial o-projection; the host sums the 4 partials per batch.

Layout notes:
- hidden is passed pre-transposed (xT [HID, S]) so projections need no
  on-chip transpose: qT/kT come out feature-major [d, s] (ready to be
  QK^T matmul operands) and v position-major [s, d] (ready for PV).
- scores are computed transposed (sT[kv, q] = k @ qT) so softmax's sum
  reduces over the partition dim via an appended ones-column in v
  (flash-attention style), and probs feed PV directly as lhsT without
  any transposes.
- matmul operands are bf16 (PSUM accumulation f32); RoPE and softmax
  normalization are done in f32.
- work is emitted head-pair-interleaved: attention for heads 0-1 overlaps
  the projections/RoPE for heads 2-3 on the PE, hiding ACT(exp) stalls.
"""
import numpy as np

B, S, H, D = 2, 2048, 16, 64
HID = H * D
HPC = 4          # heads per core
COLS = HPC * D   # 256 feature columns per core
SCALE = 0.125    # 1/sqrt(D)
NCORES = 8
VW = D + 1       # v columns + ones column

_compiled = None


def _build():
    import concourse.bacc as bacc
    import concourse.mybir as mybir
    import concourse.tile as tile

    F32 = mybir.dt.float32
    BF16 = mybir.dt.bfloat16
    Exp = mybir.ActivationFunctionType.Exp

    nc = bacc.Bacc(None)

    xT_d = nc.declare_dram_parameter("xT", [HID, S], BF16, isOutput=False)
    wq_d = nc.declare_dram_parameter("wq", [HID, COLS], BF16, isOutput=False)
    wk_d = nc.declare_dram_parameter("wk", [HID, COLS], BF16, isOutput=False)
    wv_d = nc.declare_dram_parameter("wv", [HID, COLS], BF16, isOutput=False)
    wo_d = nc.declare_dram_parameter("wo", [COLS, HID], BF16, isOutput=False)
    info_d = nc.declare_dram_parameter("infoT", [COLS, S], F32, isOutput=False)
    cos_d = nc.declare_dram_parameter("cos2", [128, S], F32, isOutput=False)
    sin_d = nc.declare_dram_parameter("sin2", [128, S], F32, isOutput=False)
    mask_d = nc.declare_dram_parameter("mask", [4, 128, 512], BF16, isOutput=False)
    vone_d = nc.declare_dram_parameter("voned", [128, 16 * HPC], BF16, isOutput=False)
    out_d = nc.declare_dram_parameter("out", [S, HID], F32, isOutput=True)

    NS = S // 512    # 4  q-tiles of 512
    NB = S // 128    # 16 kv-blocks of 128
    NK = HID // 128  # 8 contraction chunks

    with tile.TileContext(nc) as tc, \
         nc.allow_low_precision(reason="bf16 matmul operands are intentional"), \
         tc.tile_pool(name="oTpool", bufs=1) as oTp:
        oT = [oTp.tile([128, S], BF16, tag=f"oT{t}", name=f"oT{t}") for t in range(2)]
        with tc.tile_pool(name="persist", bufs=1) as pp, \
             tc.tile_pool(name="qwp", bufs=2) as qwp, \
             tc.tile_pool(name="rotp", bufs=1) as rotp, \
             tc.tile_pool(name="ptp", bufs=3) as ptp, \
             tc.tile_pool(name="oup", bufs=2) as oup, \
             tc.tile_pool(name="rip", bufs=1) as rip, \
             tc.tile_pool(name="rbp", bufs=2) as rbp:

            qT = [pp.tile([128, S], BF16, tag=f"qT{t}", name=f"qT{t}") for t in range(2)]
            kT = [pp.tile([128, S], BF16, tag=f"kT{t}", name=f"kT{t}") for t in range(2)]
            v_sb = pp.tile([128, NB * HPC * VW], BF16, tag="v_sb")
            mask_sb = pp.tile([128, 4 * 512], BF16, tag="mask_sb")

            nc.sync.dma_start(
                mask_sb[:].rearrange("p (r q) -> p r q", r=4),
                mask_d[:].rearrange("r p q -> p r q"),
            )
            nc.sync.dma_start(
                v_sb[:].rearrange("p (j h x) -> p j h x", j=NB, h=HPC)[:, :, :, D : D + 1],
                vone_d[:].rearrange("p (j h) -> p j h", j=NB)[:, :, :, None],
            )
            xT_sb = [pp.tile([128, S], BF16, tag=f"xT{kc}", name=f"xTsb{kc}") for kc in range(NK)]
            for kc in range(NK):
                nc.sync.dma_start(xT_sb[kc][:], xT_d[kc * 128 : (kc + 1) * 128, :])
            wq_sb = pp.tile([128, NK * COLS], BF16, tag="wq_sb")
            wk_sb = pp.tile([128, NK * COLS], BF16, tag="wk_sb")
            wv_sb = pp.tile([128, NK * COLS], BF16, tag="wv_sb")
            for w_sb, w_d in ((wq_sb, wq_d), (wk_sb, wk_d), (wv_sb, wv_d)):
                nc.sync.dma_start(
                    w_sb[:].rearrange("p (c n) -> p c n", c=NK),
                    w_d[:].rearrange("(c p) n -> p c n", p=128),
                )
            info_sb = [pp.tile([128, S], F32, tag=f"info{t}", name=f"info{t}") for t in range(2)]
            for t in range(2):
                nc.sync.dma_start(info_sb[t][:], info_d[t * 128 : (t + 1) * 128, :])
            cos_sb = pp.tile([128, S], F32, tag="cos_sb")
            sin_sb = pp.tile([128, S], F32, tag="sin_sb")
            nc.sync.dma_start(cos_sb[:], cos_d[:])
            nc.sync.dma_start(sin_sb[:], sin_d[:])

            pools = {}

            def proj_rope(t):
                """q/k projection + RoPE for head-pair t (-> qT[t], kT[t])."""
                qw = qwp.tile([128, S], F32, tag="qw", name=f"qwork{t}")
                kw = qwp.tile([128, S], F32, tag="qw", name=f"kwork{t}")
                for w_sb, wk_ in ((wq_sb, qw), (wk_sb, kw)):
                    for n in range(NS):
                        ps = pools["ppqk"].tile([128, 512], F32, tag="pqk", name=f"pqk{t}{n}")
                        for kc in range(NK):
                            nc.tensor.matmul(
                                ps[:],
                                w_sb[:, kc * COLS + t * 128 : kc * COLS + (t + 1) * 128],
                                xT_sb[kc][:, n * 512 : (n + 1) * 512],
                                start=(kc == 0),
                                stop=(kc == NK - 1),
                            )
                        nc.vector.tensor_add(
                            wk_[:, n * 512 : (n + 1) * 512],
                            ps[:],
                            info_sb[t][:, n * 512 : (n + 1) * 512],
                        )
                for wk_, dst in ((qw, qT), (kw, kT)):
                    rt = rotp.tile([128, S], F32, tag="rt", name=f"rt{t}")
                    for hb in range(2):
                        b0 = 64 * hb
                        nc.gpsimd.tensor_scalar_mul(
                            rt[b0 : b0 + 32, :], wk_[b0 + 32 : b0 + 64, :], -1.0
                        )
                        nc.gpsimd.tensor_copy(
                            rt[b0 + 32 : b0 + 64, :], wk_[b0 : b0 + 32, :]
                        )
                    nc.vector.tensor_mul(wk_[:], wk_[:], cos_sb[:])
                    nc.vector.tensor_mul(rt[:], rt[:], sin_sb[:])
                    nc.vector.tensor_add(dst[t][:], wk_[:], rt[:])

            def vproj(hp):
                """v projection, all 4 heads (hp ignored; called once)."""
                for sb in range(NB):
                    ps = pools["ppqk"].tile([128, COLS], F32, tag="pqk", name=f"pv{sb}")
                    for kc in range(NK):
                        nc.tensor.matmul(
                            ps[:],
                            xT_sb[kc][:, sb * 128 : (sb + 1) * 128],
                            wv_sb[:, kc * COLS : (kc + 1) * COLS],
                            start=(kc == 0),
                            stop=(kc == NK - 1),
                        )
                    nc.scalar.copy(
                        v_sb[:, sb * HPC * VW : (sb + 1) * HPC * VW]
                        .rearrange("p (h x) -> p h x", h=HPC)[:, :, 0:D],
                        ps[:].rearrange("p (h x) -> p h x", h=HPC),
                    )

            def attn(hp):
                """causal attention for head-pair hp (-> oT[hp], bf16)."""
                oun = [
                    oup.tile([VW, S], F32, tag="oun", name=f"oun{hp}{hh}")
                    for hh in range(2)
                ]
                for ib in range(NS):
                    o_ps = [
                        pools["ppo"].tile([VW, 512], F32, tag="o_ps", name=f"ops{hp}{ib}{hh}")
                        for hh in range(2)
                    ]
                    njb = 4 * (ib + 1)
                    for jbp in range(njb // 2):
                        diag = jbp // 2 == ib
                        sc = [
                            pools["ppsc"].tile([128, 1024], F32, tag="sc", name=f"sc{hp}{ib}{jbp}{hh}")
                            for hh in range(2)
                        ]
                        for hh in range(2):
                            b0 = 64 * hh
                            for sub in range(2):
                                jb = 2 * jbp + sub
                                nc.tensor.matmul(
                                    sc[hh][:, sub * 512 : (sub + 1) * 512],
                                    kT[hp][b0 : b0 + 64, jb * 128 : (jb + 1) * 128],
                                    qT[hp][b0 : b0 + 64, ib * 512 : (ib + 1) * 512],
                                    start=True,
                                    stop=True,
                                )
                        for hh in range(2):
                            pt = ptp.tile([128, 1024], BF16, tag="pt", name=f"pt{hp}{ib}{jbp}{hh}")
                            nc.scalar.activation(pt[:], sc[hh][:], Exp, scale=SCALE)
                            if diag:
                                for sub in range(2):
                                    jb = 2 * jbp + sub
                                    r = jb % 4
                                    nc.gpsimd.tensor_mul(
                                        pt[:, sub * 512 : (sub + 1) * 512],
                                        pt[:, sub * 512 : (sub + 1) * 512],
                                        mask_sb[:, r * 512 : (r + 1) * 512],
                                    )
                            h = 2 * hp + hh
                            for sub in range(2):
                                jb = 2 * jbp + sub
                                nc.tensor.matmul(
                                    o_ps[hh][:],
                                    v_sb[:, (jb * HPC + h % HPC) * VW : (jb * HPC + h % HPC) * VW + VW],
                                    pt[:, sub * 512 : (sub + 1) * 512],
                                    start=(jb == 0),
                                    stop=(jb == njb - 1),
                                )
                    for hh in range(2):
                        nc.vector.tensor_copy(
                            oun[hh][:, ib * 512 : (ib + 1) * 512], o_ps[hh][:]
                        )
                # batched softmax normalization per head
                for hh in range(2):
                    lb = rip.tile([1, S], F32, tag="lb", name=f"lb{hp}{hh}")
                    nc.vector.tensor_copy(lb[:], oun[hh][D : D + 1, :])
                    ri = rip.tile([1, S], F32, tag="ri", name=f"ri{hp}{hh}")
                    nc.vector.reciprocal_approx_fast(ri[:], lb[:])
                    rb = rbp.tile([64, S], F32, tag="rb", name=f"rb{hp}{hh}")
                    nc.gpsimd.partition_broadcast(rb[:], ri[:])
                    nc.vector.tensor_mul(
                        oT[hp][64 * hh : 64 * hh + 64, :], oun[hh][0:D, :], rb[:]
                    )

            # ---- emission order: pipeline hp0 attention against hp1 prep ----
            with tc.tile_pool(name="ppqk", bufs=2, space="PSUM") as _ppqk, \
                 tc.tile_pool(name="ppsc", bufs=2, space="PSUM") as _ppsc, \
                 tc.tile_pool(name="ppo", bufs=2, space="PSUM") as _ppo:
                pools["ppqk"], pools["ppsc"], pools["ppo"] = _ppqk, _ppsc, _ppo
                proj_rope(0)
                vproj(0)
                attn(0)
                proj_rope(1)
                attn(1)

        # ---------------- output projection ----------------
        with tc.tile_pool(name="phC", bufs=1) as pc, \
             tc.tile_pool(name="outp", bufs=2) as outp, \
             tc.tile_pool(name="ppout", bufs=4, space="PSUM") as ppout:
            wo_sb = [pc.tile([128, HID], BF16, tag=f"wo{t}", name=f"wosb{t}") for t in range(2)]
            for t in range(2):
                nc.sync.dma_start(wo_sb[t][:], wo_d[t * 128 : (t + 1) * 128, :])
            for sb in range(NB):
                ot = outp.tile([128, HID], F32, tag="ot")
                for half in range(2):
                    ps = ppout.tile([128, 512], F32, tag="po")
                    for kh in range(2):
                        nc.tensor.matmul(
                            ps[:],
                            oT[kh][:, sb * 128 : (sb + 1) * 128],
                            wo_sb[kh][:, half * 512 : (half + 1) * 512],
                            start=(kh == 0),
                            stop=(kh == 1),
                        )
                    nc.vector.tensor_copy(ot[:, half * 512 : (half + 1) * 512], ps[:])
                nc.sync.dma_start(out_d[sb * 128 : (sb + 1) * 128, :], ot[:])

    nc.compile()
    return nc


def _prep_inputs(hidden_states, info_embeddings, Wq, Wk, Wv, Wo, position_ids):
    import ml_dtypes
    bf16 = ml_dtypes.bfloat16
    f32 = np.float32
    hid = np.asarray(hidden_states, dtype=f32)
    info = np.asarray(info_embeddings, dtype=f32)
    Wq = np.asarray(Wq, dtype=f32)
    Wk = np.asarray(Wk, dtype=f32)
    Wv = np.asarray(Wv, dtype=f32)
    Wo = np.asarray(Wo, dtype=f32)
    pos = np.asarray(position_ids)

    inv = 1.0 / (10000.0 ** (np.arange(0, D, 2, dtype=np.float64) / D))
    mask = np.zeros((4, 128, 512), dtype=bf16)
    for r in range(4):
        q_idx = np.arange(512)[None, :]
        kv_idx = np.arange(128)[:, None]
        mask[r] = (q_idx >= 128 * r + kv_idx).astype(bf16)
    voned = np.ones((128, 16 * HPC), dtype=bf16)

    in_maps = []
    for c in range(NCORES):
        b, g = divmod(c, 4)
        cols = slice(COLS * g, COLS * (g + 1))
        fr = pos[b].astype(np.float64)[:, None] * inv[None, :]
        emb = np.concatenate([fr, fr], axis=-1)            # [S, D]
        cosT = np.cos(emb).T.astype(f32)                   # [D, S]
        sinT = np.sin(emb).T.astype(f32)
        in_maps.append({
            "xT": np.ascontiguousarray(hid[b].T).astype(bf16),
            "wq": np.ascontiguousarray(Wq[:, cols]).astype(bf16),
            "wk": np.ascontiguousarray(Wk[:, cols]).astype(bf16),
            "wv": np.ascontiguousarray(Wv[:, cols]).astype(bf16),
            "wo": np.ascontiguousarray(Wo[cols, :]).astype(bf16),
            "infoT": np.ascontiguousarray(info[b].T[cols, :]),
            "cos2": np.ascontiguousarray(np.vstack([cosT, cosT])),
            "sin2": np.ascontiguousarray(np.vstack([sinT, sinT])),
            "mask": mask,
            "voned": voned,
        })
    return in_maps


def kernel(hidden_states, info_embeddings, Wq, Wk, Wv, Wo, position_ids):
    global _compiled
    from concourse.bass_utils import run_bass_kernel_spmd

    if _compiled is None:
        _compiled = _build()
    in_maps = _prep_inputs(
        hidden_states, info_embeddings, Wq, Wk, Wv, Wo, position_ids
    )
    res = run_bass_kernel_spmd(_compiled, in_maps, core_ids=list(range(NCORES)))
    parts = [r["out"] for r in res.results]
    out = np.stack(
        [np.sum(parts[4 * b : 4 * b + 4], axis=0, dtype=np.float32) for b in range(B)]
    )
    return out.astype(np.float32)


# revision 16
# speedup vs baseline: 2.2392x; 2.2392x over previous
"""CarFormer attention kernel for 8 TRN2 NeuronCores.

Sharding: data-parallel over batch (B=2), tensor-parallel over heads
(16 heads -> 4 per core). Core c handles batch b=c//4, heads
[4*(c%4), 4*(c%4)+4). Each core computes q/k/v projections for its head
slice, RoPE, causal flash-style attention (transposed-scores layout), and
a partial o-projection; the host sums the 4 partials per batch.

Layout notes:
- hidden is passed pre-transposed (xT [HID, S]) so projections need no
  on-chip transpose: qT/kT come out feature-major [d, s] (ready to be
  QK^T matmul operands) and v position-major [s, d] (ready for PV).
- scores are computed transposed (sT[kv, q] = k @ qT) so softmax's sum
  reduces over the partition dim via an appended ones-column in v
  (flash-attention style), and probs feed PV directly as lhsT without
  any transposes.
- matmul operands are bf16 (PSUM accumulation f32); RoPE and softmax
  normalization are done in f32.
- work is emitted head-pair-interleaved: attention for heads 0-1 overlaps
  the projections/RoPE for heads 2-3 on the PE, hiding ACT(exp) stalls.
"""
import numpy as np

B, S, H, D = 2, 2048, 16, 64
HID = H * D
HPC = 4          # heads per core
COLS = HPC * D   # 256 feature columns per core
SCALE = 0.125    # 1/sqrt(D)
NCORES = 8
VW = D + 1       # v columns + ones column

_compiled = None


def _build():
    import concourse.bacc as bacc
    import concourse.mybir as mybir
    import concourse.tile as tile

    F32 = mybir.dt.float32
    BF16 = mybir.dt.bfloat16
    Exp = mybir.ActivationFunctionType.Exp

    nc = bacc.Bacc(None)

    xT_d = nc.declare_dram_parameter("xT", [HID, S], BF16, isOutput=False)
    wq_d = nc.declare_dram_parameter("wq", [HID, COLS], BF16, isOutput=False)
    wk_d = nc.declare_dram_parameter("wk", [HID, COLS], BF16, isOutput=False)
    wv_d = nc.declare_dram_parameter("wv", [HID, COLS], BF16, isOutput=False)
    wo_d = nc.declare_dram_parameter("wo", [COLS, HID], BF16, isOutput=False)
    info_d = nc.declare_dram_parameter("infoT", [COLS, S], F32, isOutput=False)
    cos_d = nc.declare_dram_parameter("cos2", [128, S], F32, isOutput=False)
    sin_d = nc.declare_dram_parameter("sin2", [128, S], F32, isOutput=False)
    mask_d = nc.declare_dram_parameter("mask", [4, 128, 512], BF16, isOutput=False)
    vone_d = nc.declare_dram_parameter("voned", [128, 16 * HPC], BF16, isOutput=False)
    out_d = nc.declare_dram_parameter("out", [S, HID], F32, isOutput=True)

    NS = S // 512    # 4  q-tiles of 512
    NB = S // 128    # 16 kv-blocks of 128
    NK = HID // 128  # 8 contraction chunks

    with tile.TileContext(nc) as tc, \
         nc.allow_low_precision(reason="bf16 matmul operands are intentional"), \
         tc.tile_pool(name="oTpool", bufs=1) as oTp:
        oT = [oTp.tile([128, S], BF16, tag=f"oT{t}", name=f"oT{t}") for t in range(2)]
        with tc.tile_pool(name="persist", bufs=1) as pp, \
             tc.tile_pool(name="qwp", bufs=2) as qwp, \
             tc.tile_pool(name="rotp", bufs=1) as rotp, \
             tc.tile_pool(name="ptp", bufs=3) as ptp, \
             tc.tile_pool(name="oup", bufs=2) as oup, \
             tc.tile_pool(name="rip", bufs=1) as rip, \
             tc.tile_pool(name="rbp", bufs=2) as rbp:

            qT = [pp.tile([128, S], BF16, tag=f"qT{t}", name=f"qT{t}") for t in range(2)]
            kT = [pp.tile([128, S], BF16, tag=f"kT{t}", name=f"kT{t}") for t in range(2)]
            v_sb = pp.tile([128, NB * HPC * VW], BF16, tag="v_sb")
            mask_sb = pp.tile([128, 4 * 512], BF16, tag="mask_sb")

            nc.sync.dma_start(
                mask_sb[:].rearrange("p (r q) -> p r q", r=4),
                mask_d[:].rearrange("r p q -> p r q"),
            )
            nc.sync.dma_start(
                v_sb[:].rearrange("p (j h x) -> p j h x", j=NB, h=HPC)[:, :, :, D : D + 1],
                vone_d[:].rearrange("p (j h) -> p j h", j=NB)[:, :, :, None],
            )
            xT_sb = [pp.tile([128, S], BF16, tag=f"xT{kc}", name=f"xTsb{kc}") for kc in range(NK)]
            for kc in range(NK):
                nc.sync.dma_start(xT_sb[kc][:], xT_d[kc * 128 : (kc + 1) * 128, :])
            wq_sb = pp.tile([128, NK * COLS], BF16, tag="wq_sb")
            wk_sb = pp.tile([128, NK * COLS], BF16, tag="wk_sb")
            wv_sb = pp.tile([128, NK * COLS], BF16, tag="wv_sb")
            for w_sb, w_d in ((wq_sb, wq_d), (wk_sb, wk_d), (wv_sb, wv_d)):
                nc.sync.dma_start(
                    w_sb[:].rearrange("p (c n) -> p c n", c=NK),
                    w_d[:].rearrange("(c p) n -> p c n", p=128),
                )
            info_sb = [pp.tile([128, S], F32, tag=f"info{t}", name=f"info{t}") for t in range(2)]
            for t in range(2):
                nc.sync.dma_start(info_sb[t][:], info_d[t * 128 : (t + 1) * 128, :])
            cos_sb = pp.tile([128, S], F32, tag="cos_sb")
            sin_sb = pp.tile([128, S], F32, tag="sin_sb")
            nc.sync.dma_start(cos_sb[:], cos_d[:])
            nc.sync.dma_start(sin_sb[:], sin_d[:])

            pools = {}

            def proj_rope(t):
                """q/k projection + RoPE for head-pair t (-> qT[t], kT[t])."""
                qw = qwp.tile([128, S], F32, tag="qw", name=f"qwork{t}")
                kw = qwp.tile([128, S], F32, tag="qw", name=f"kwork{t}")
                for w_sb, wk_ in ((wq_sb, qw), (wk_sb, kw)):
                    for n in range(NS):
                        ps = pools["ppqk"].tile([128, 512], F32, tag="pqk", name=f"pqk{t}{n}")
                        for kc in range(NK):
                            nc.tensor.matmul(
                                ps[:],
                                w_sb[:, kc * COLS + t * 128 : kc * COLS + (t + 1) * 128],
                                xT_sb[kc][:, n * 512 : (n + 1) * 512],
                                start=(kc == 0),
                                stop=(kc == NK - 1),
                            )
                        nc.vector.tensor_add(
                            wk_[:, n * 512 : (n + 1) * 512],
                            ps[:],
                            info_sb[t][:, n * 512 : (n + 1) * 512],
                        )
                for wk_, dst in ((qw, qT), (kw, kT)):
                    rt = rotp.tile([128, S], F32, tag="rt", name=f"rt{t}")
                    for hb in range(2):
                        b0 = 64 * hb
                        nc.vector.tensor_scalar_mul(
                            rt[b0 : b0 + 32, :], wk_[b0 + 32 : b0 + 64, :], -1.0
                        )
                        nc.vector.tensor_copy(
                            rt[b0 + 32 : b0 + 64, :], wk_[b0 : b0 + 32, :]
                        )
                    nc.vector.tensor_mul(wk_[:], wk_[:], cos_sb[:])
                    nc.vector.tensor_mul(rt[:], rt[:], sin_sb[:])
                    nc.vector.tensor_add(dst[t][:], wk_[:], rt[:])

            def vproj(hp):
                """v projection, all 4 heads (hp ignored; called once)."""
                for sb in range(NB):
                    ps = pools["ppqk"].tile([128, COLS], F32, tag="pqk", name=f"pv{sb}")
                    for kc in range(NK):
                        nc.tensor.matmul(
                            ps[:],
                            xT_sb[kc][:, sb * 128 : (sb + 1) * 128],
                            wv_sb[:, kc * COLS : (kc + 1) * COLS],
                            start=(kc == 0),
                            stop=(kc == NK - 1),
                        )
                    nc.scalar.copy(
                        v_sb[:, sb * HPC * VW : (sb + 1) * HPC * VW]
                        .rearrange("p (h x) -> p h x", h=HPC)[:, :, 0:D],
                        ps[:].rearrange("p (h x) -> p h x", h=HPC),
                    )

            def attn(hp):
                """causal attention for head-pair hp (-> oT[hp], bf16)."""
                oun = [
                    oup.tile([VW, S], F32, tag="oun", name=f"oun{hp}{hh}")
                    for hh in range(2)
                ]
                for ib in range(NS):
                    o_ps = [
                        pools["ppo"].tile([VW, 512], F32, tag="o_ps", name=f"ops{hp}{ib}{hh}")
                        for hh in range(2)
                    ]
                    njb = 4 * (ib + 1)
                    for jbp in range(njb // 2):
                        diag = jbp // 2 == ib
                        sc = [
                            pools["ppsc"].tile([128, 1024], F32, tag="sc", name=f"sc{hp}{ib}{jbp}{hh}")
                            for hh in range(2)
                        ]
                        for hh in range(2):
                            b0 = 64 * hh
                            for sub in range(2):
                                jb = 2 * jbp + sub
                                nc.tensor.matmul(
                                    sc[hh][:, sub * 512 : (sub + 1) * 512],
                                    kT[hp][b0 : b0 + 64, jb * 128 : (jb + 1) * 128],
                                    qT[hp][b0 : b0 + 64, ib * 512 : (ib + 1) * 512],
                                    start=True,
                                    stop=True,
                                )
                        for hh in range(2):
                            pt = ptp.tile([128, 1024], BF16, tag="pt", name=f"pt{hp}{ib}{jbp}{hh}")
                            nc.scalar.activation(pt[:], sc[hh][:], Exp, scale=SCALE)
                            if diag:
                                for sub in range(2):
                                    jb = 2 * jbp + sub
                                    r = jb % 4
                                    nc.vector.tensor_mul(
                                        pt[:, sub * 512 : (sub + 1) * 512],
                                        pt[:, sub * 512 : (sub + 1) * 512],
                                        mask_sb[:, r * 512 : (r + 1) * 512],
                                    )
                            h = 2 * hp + hh
                            for sub in range(2):
                                jb = 2 * jbp + sub
                                nc.tensor.matmul(
                                    o_ps[hh][:],
                                    v_sb[:, (jb * HPC + h % HPC) * VW : (jb * HPC + h % HPC) * VW + VW],
                                    pt[:, sub * 512 : (sub + 1) * 512],
                                    start=(jb == 0),
                                    stop=(jb == njb - 1),
                                )
                    for hh in range(2):
                        nc.vector.tensor_copy(
                            oun[hh][:, ib * 512 : (ib + 1) * 512], o_ps[hh][:]
                        )
                # batched softmax normalization per head
                for hh in range(2):
                    lb = rip.tile([1, S], F32, tag="lb", name=f"lb{hp}{hh}")
                    nc.vector.tensor_copy(lb[:], oun[hh][D : D + 1, :])
                    ri = rip.tile([1, S], F32, tag="ri", name=f"ri{hp}{hh}")
                    nc.vector.reciprocal_approx_fast(ri[:], lb[:])
                    rb = rbp.tile([64, S], F32, tag="rb", name=f"rb{hp}{hh}")
                    nc.gpsimd.partition_broadcast(rb[:], ri[:])
                    nc.vector.tensor_mul(
                        oT[hp][64 * hh : 64 * hh + 64, :], oun[hh][0:D, :], rb[:]
                    )

            # ---- emission order: pipeline hp0 attention against hp1 prep ----
            with tc.tile_pool(name="ppqk", bufs=2, space="PSUM") as _ppqk, \
                 tc.tile_pool(name="ppsc", bufs=2, space="PSUM") as _ppsc, \
                 tc.tile_pool(name="ppo", bufs=2, space="PSUM") as _ppo:
                pools["ppqk"], pools["ppsc"], pools["ppo"] = _ppqk, _ppsc, _ppo
                proj_rope(0)
                vproj(0)
                attn(0)
                proj_rope(1)
                attn(1)

        # ---------------- output projection ----------------
        with tc.tile_pool(name="phC", bufs=1) as pc, \
             tc.tile_pool(name="outp", bufs=2) as outp, \
             tc.tile_pool(name="ppout", bufs=4, space="PSUM") as ppout:
            wo_sb = [pc.tile([128, HID], BF16, tag=f"wo{t}", name=f"wosb{t}") for t in range(2)]
            for t in range(2):
                nc.sync.dma_start(wo_sb[t][:], wo_d[t * 128 : (t + 1) * 128, :])
            for sb in range(NB):
                ot = outp.tile([128, HID], F32, tag="ot")
                for half in range(2):
                    ps = ppout.tile([128, 512], F32, tag="po")
                    for kh in range(2):
                        nc.tensor.matmul(
                            ps[:],
                            oT[kh][:, sb * 128 : (sb + 1) * 128],
                            wo_sb[kh][:, half * 512 : (half + 1) * 512],
                            start=(kh == 0),
                            stop=(kh == 1),
                        )
                    nc.vector.tensor_copy(ot[:, half * 512 : (half + 1) * 512], ps[:])
                nc.sync.dma_start(out_d[sb * 128 : (sb + 1) * 128, :], ot[:])

    nc.compile()
    return nc


def _prep_inputs(hidden_states, info_embeddings, Wq, Wk, Wv, Wo, position_ids):
    import ml_dtypes
    bf16 = ml_dtypes.bfloat16
    f32 = np.float32
    hid = np.asarray(hidden_states, dtype=f32)
    info = np.asarray(info_embeddings, dtype=f32)
    Wq = np.asarray(Wq, dtype=f32)
    Wk = np.asarray(Wk, dtype=f32)
    Wv = np.asarray(Wv, dtype=f32)
    Wo = np.asarray(Wo, dtype=f32)
    pos = np.asarray(position_ids)

    inv = 1.0 / (10000.0 ** (np.arange(0, D, 2, dtype=np.float64) / D))
    mask = np.zeros((4, 128, 512), dtype=bf16)
    for r in range(4):
        q_idx = np.arange(512)[None, :]
        kv_idx = np.arange(128)[:, None]
        mask[r] = (q_idx >= 128 * r + kv_idx).astype(bf16)
    voned = np.ones((128, 16 * HPC), dtype=bf16)

    in_maps = []
    for c in range(NCORES):
        b, g = divmod(c, 4)
        cols = slice(COLS * g, COLS * (g + 1))
        fr = pos[b].astype(np.float64)[:, None] * inv[None, :]
        emb = np.concatenate([fr, fr], axis=-1)            # [S, D]
        cosT = np.cos(emb).T.astype(f32)                   # [D, S]
        sinT = np.sin(emb).T.astype(f32)
        in_maps.append({
            "xT": np.ascontiguousarray(hid[b].T).astype(bf16),
            "wq": np.ascontiguousarray(Wq[:, cols]).astype(bf16),
            "wk": np.ascontiguousarray(Wk[:, cols]).astype(bf16),
            "wv": np.ascontiguousarray(Wv[:, cols]).astype(bf16),
            "wo": np.ascontiguousarray(Wo[cols, :]).astype(bf16),
            "infoT": np.ascontiguousarray(info[b].T[cols, :]),
            "cos2": np.ascontiguousarray(np.vstack([cosT, cosT])),
            "sin2": np.ascontiguousarray(np.vstack([sinT, sinT])),
            "mask": mask,
            "voned": voned,
        })
    return in_maps


def kernel(hidden_states, info_embeddings, Wq, Wk, Wv, Wo, position_ids):
    global _compiled
    from concourse.bass_utils import run_bass_kernel_spmd

    if _compiled is None:
        _compiled = _build()
    in_maps = _prep_inputs(
        hidden_states, info_embeddings, Wq, Wk, Wv, Wo, position_ids
    )
    res = run_bass_kernel_spmd(_compiled, in_maps, core_ids=list(range(NCORES)))
    parts = [r["out"] for r in res.results]
    out = np.stack(
        [np.sum(parts[4 * b : 4 * b + 4], axis=0, dtype=np.float32) for b in range(B)]
    )
    return out.astype(np.float32)
